# revision 18
# baseline (speedup 1.0000x reference)
"""Trainium2 Bass kernel for nn_Channel_Seq_Big_Attention.

Reference computation (per batch b of 8, fully data-parallel across 8 cores):
  x: (N=128, M=8, D=512) tokens; q = x@w_q, k,v = x@w_kv (INNER=512, H=8, DH=64)
  sim[i,j,m,z] = q[i,m]·k[j,z] * DH**-0.5     (cross-seq, cross-modality)
  attn = softmax over the QUERY-seq dim i (axis 2 of (b,h,i,j,m,z))
  out[i,z,d] = sum_{j,m} attn[i,j,m,z] v[j,m,d]
  y = out.reshape(N, M*H*DH) @ w_out + b_out   (col order z*H*DH + h*DH + d)

Because softmax normalizes over i (not the contracted j), the denominator
L[j,m,z] = sum_i exp(S[i,j,m,z]) folds into V:
  out_z = sum_m exp(S_mz) @ (v_m / L_mz[j])
Device dataflow per core (tokens kept in modality-major order m*N+j):
  - host pre-transposes/casts x to xT (D, T) bf16; projections contract D on
    partitions producing qT/kT ((h dh), T) directly.
  - S^T tiles (keys j on partitions, queries (m,i) free) via matmul
    lhsT=k-block, rhs=qT -> exp on ScalarE (PSUM->SBUF bf16) -> row sums per
    query-modality segment on VectorE -> reciprocal -> scale v by Linv
    (per-partition j scalars) -> PV matmuls (lhsT=v-scaled, rhs=exp(S^T))
    accumulate over m in PSUM -> out projection streamed from HBM.
"""

import sys

import numpy as np

for _p in ("/opt/trn_rl_repo",):
    if _p not in sys.path:
        sys.path.insert(0, _p)

import ml_dtypes  # noqa: E402

B, N, M, D = 8, 128, 8, 512
H, DH = 8, 64
INNER = H * DH          # 512
T = N * M               # 1024 tokens per batch element
CD = INNER * M          # 4096 contraction dim of out projection
NCORES = 8

BF16 = ml_dtypes.bfloat16

_CACHE = {}


def build_nc():
    import concourse.bass as bass
    import concourse.mybir as mybir
    import concourse.tile as tile
    from concourse import bacc

    fp32 = mybir.dt.float32
    bf16 = mybir.dt.bfloat16

    nc = bacc.Bacc(trn_type="TRN2", target_bir_lowering=False, debug=False)

    xT = nc.dram_tensor("xT", (D, T), bf16, kind="ExternalInput").ap()
    w_q = nc.dram_tensor("w_q", (D, INNER), bf16, kind="ExternalInput").ap()
    w_k = nc.dram_tensor("w_k", (D, INNER), bf16, kind="ExternalInput").ap()
    w_v = nc.dram_tensor("w_v", (D, INNER), bf16, kind="ExternalInput").ap()
    w_out = nc.dram_tensor("w_out", (CD, CD), bf16, kind="ExternalInput").ap()
    y = nc.dram_tensor("y", (N, CD), fp32, kind="ExternalOutput").ap()

    KC = D // 128        # 4 contraction chunks for the projections
    PC = INNER // 128    # 4 partition chunks of qT/kT
    SCALE = DH ** -0.5

    with tile.TileContext(nc) as tc:
        with tc.tile_pool(name="persist", bufs=1) as persist:
            xT_sb = persist.tile([128, KC, T], bf16)
            wq_sb = persist.tile([128, KC, INNER], bf16)
            wk_sb = persist.tile([128, KC, INNER], bf16)
            wv_sb = persist.tile([128, KC, INNER], bf16)
            qT_sb = persist.tile([128, PC, T], bf16)
            kT_sb = persist.tile([128, PC, T], bf16)
            v_sb = persist.tile([128, M, INNER], bf16)
            ofT_sb = persist.tile([128, CD // 128, N], bf16)
            y_sb = persist.tile([128, CD], fp32)

            nc.sync.dma_start(xT_sb[:], xT.rearrange("(c p) t -> p c t", p=128))
            nc.sync.dma_start(wq_sb[:], w_q.rearrange("(c p) n -> p c n", p=128))
            nc.sync.dma_start(wk_sb[:], w_k.rearrange("(c p) n -> p c n", p=128))
            nc.sync.dma_start(wv_sb[:], w_v.rearrange("(c p) n -> p c n", p=128))

            # w_out stream: pool opened before the attention pools so it gets
            # its own SBUF range (no address reuse -> DMAs prefetch from t=0,
            # gated only by slot recycling).
            NKC = CD // 128  # 32
            wo_pool = tc.alloc_tile_pool(name="wo_pool", bufs=12)
            wo_tiles = []
            for kc in range(NKC):
                wo_t = wo_pool.tile([128, CD], bf16, name="wo_t", tag="wo")
                nc.sync.dma_start(wo_t[:], w_out[kc * 128:(kc + 1) * 128, :])
                wo_tiles.append(wo_t)

            # ---- projections: qT/kT ((h dh) on partitions, tokens free), v ----
            with tc.tile_pool(name="proj_psum", bufs=4, space="PSUM") as proj_psum:
                for dst, w_sb in ((qT_sb, wq_sb), (kT_sb, wk_sb)):
                    for pc in range(PC):
                        for th in range(T // 512):
                            pj = proj_psum.tile([128, 512], fp32, name="pj", tag="pj")
                            for kc in range(KC):
                                nc.tensor.matmul(
                                    pj[:],
                                    w_sb[:, kc, pc * 128:(pc + 1) * 128],
                                    xT_sb[:, kc, th * 512:(th + 1) * 512],
                                    start=(kc == 0),
                                    stop=(kc == KC - 1),
                                )
                            nc.scalar.copy(dst[:, pc, th * 512:(th + 1) * 512], pj[:])
                for m in range(M):
                    pj = proj_psum.tile([128, 512], fp32, name="pj", tag="pj")
                    for kc in range(KC):
                        nc.tensor.matmul(
                            pj[:],
                            xT_sb[:, kc, m * 128:(m + 1) * 128],
                            wv_sb[:, kc, :],
                            start=(kc == 0),
                            stop=(kc == KC - 1),
                        )
                    nc.scalar.copy(v_sb[:, m, :], pj[:])

            # ---- attention ----
            # Heads are processed in pairs (2g, 2g+1).  The two heads' sim
            # matmuls use K row-groups 0-63 / 64-127 and their PV matmuls use
            # output col-groups 0-63 / 64-127, so interleaving them lets the
            # PE array run both concurrently (~2x).
            with (
                tc.tile_pool(name="sim_psum", bufs=3, space="PSUM") as sim_psum,
                tc.tile_pool(name="pv_psum", bufs=1, space="PSUM") as pv_psum,
                tc.tile_pool(name="p_pool", bufs=4) as p_pool,
                tc.tile_pool(name="vt_pool", bufs=6) as vt_pool,
                tc.tile_pool(name="stat_pool", bufs=8) as stat_pool,
            ):
                for g in range(H // 2):  # head pairs
                    opv = pv_psum.tile([128, M * 128], fp32, name="opv", tag="opv")
                    hc = g
                    q0, k0 = qT_sb[0:64, hc, :], kT_sb[0:64, hc, :]
                    q1, k1 = qT_sb[64:128, hc, :], kT_sb[64:128, hc, :]
                    for z in range(M):
                        # S^T_z per head: keys (z,j) on partitions, (m,i) free
                        ps0 = sim_psum.tile([128, T], fp32, name="ps0", tag="ps")
                        ps1 = sim_psum.tile([128, T], fp32, name="ps1", tag="ps")
                        for th in range(T // 512):
                            sl = bass.ts(th, 512)
                            nc.tensor.matmul(
                                ps0[:, sl], k0[:, bass.ts(z, 128)], q0[:, sl],
                                start=True, stop=True,
                            )
                            nc.tensor.matmul(
                                ps1[:, sl], k1[:, bass.ts(z, 128)], q1[:, sl],
                                start=True, stop=True,
                            )
                        # exp into one pair tile: [j, (h, m, i)], and
                        # L[j, (h, m)] = sum_i P^T[j, (h, m, i)].
                        # Most tiles: 2 big exps on ScalarE + segmented reduce
                        # on VectorE.  A fraction instead fuse the sums into
                        # 16 small exps via accum_out (ScalarE), offloading
                        # the saturated VectorE.
                        p_sb = p_pool.tile([128, 2 * T], bf16, name="p_sb", tag="p")
                        lsum = stat_pool.tile([128, 2 * M], fp32, name="lsum", tag="ls")
                        if False:  # ACT-fused sums: READ_ACCUMULATOR makes it a net loss
                            for hh, psx in ((0, ps0), (1, ps1)):
                                for m in range(M):
                                    seg = hh * T + m * 128
                                    nc.scalar.activation(
                                        p_sb[:, seg:seg + 128],
                                        psx[:, m * 128:(m + 1) * 128],
                                        mybir.ActivationFunctionType.Exp,
                                        scale=SCALE,
                                        accum_out=lsum[:, hh * M + m:hh * M + m + 1],
                                    )
                        else:
                            nc.scalar.activation(
                                p_sb[:, 0:T], ps0[:],
                                mybir.ActivationFunctionType.Exp, scale=SCALE,
                            )
                            nc.scalar.activation(
                                p_sb[:, T:2 * T], ps1[:],
                                mybir.ActivationFunctionType.Exp, scale=SCALE,
                            )
                            nc.vector.tensor_reduce(
                                lsum[:],
                                p_sb[:].rearrange("p (hm i) -> p hm i", i=128),
                                axis=mybir.AxisListType.X,
                                op=mybir.AluOpType.add,
                            )
                        linv = stat_pool.tile([128, 2 * M], fp32, name="linv", tag="li")
                        nc.vector.reciprocal(linv[:], lsum[:])
                        # vt[j, m, h, d] = v[j, m, (pair cols)] * Linv[j, (h, m)]
                        # on GpSimd -- otherwise idle, frees VectorE.
                        vt = vt_pool.tile([128, M, 2, DH], bf16, name="vt", tag="vt")
                        nc.gpsimd.tensor_tensor(
                            vt[:],
                            v_sb[:, :, g * 128:(g + 1) * 128].rearrange(
                                "p m (h d) -> p m h d", h=2
                            ),
                            linv[:].rearrange("p (h m) -> p m h", h=2)
                            .unsqueeze(3).broadcast_to((128, M, 2, DH)),
                            op=mybir.AluOpType.mult,
                        )
                        # PV: one accumulation group per head per z-region
                        # (groups in the same PSUM zero region must not
                        # interleave their start/stop windows).
                        for hh in range(2):
                            for m in range(M):
                                nc.tensor.matmul(
                                    opv[hh * 64:hh * 64 + 64, bass.ts(z, 128)],
                                    vt[:, m, hh, :],
                                    p_sb[:, hh * T + m * 128:hh * T + (m + 1) * 128],
                                    start=(m == 0),
                                    stop=(m == M - 1),
                                )
                    nc.vector.tensor_copy(
                        ofT_sb[:, g::4, :],
                        opv[:].rearrange("p (z i) -> p z i", i=128),
                    )

            # ---- out projection: y = out_flat @ w_out ----
            with tc.tile_pool(name="y_psum", bufs=1, space="PSUM") as y_psum:
                NB = CD // 512   # 8
                yps = [
                    y_psum.tile([128, 512], fp32, name=f"yp{nb}", tag=f"yp{nb}")
                    for nb in range(NB)
                ]
                for kc in range(NKC):
                    wo_t = wo_tiles[kc]
                    for nb in range(NB):
                        nc.tensor.matmul(
                            yps[nb][:],
                            ofT_sb[:, kc, :],
                            wo_t[:, nb * 512:(nb + 1) * 512],
                            start=(kc == 0),
                            stop=(kc == NKC - 1),
                        )
                for nb in range(NB):
                    nc.vector.tensor_copy(
                        y_sb[:, nb * 512:(nb + 1) * 512], yps[nb][:]
                    )
                    if nb % 2 == 1:
                        nc.sync.dma_start(
                            y[:, (nb - 1) * 512:(nb + 1) * 512],
                            y_sb[:, (nb - 1) * 512:(nb + 1) * 512],
                        )
            wo_pool.release()

    nc.compile()
    return nc


def _get_nc():
    if "nc" not in _CACHE:
        _CACHE["nc"] = build_nc()
    return _CACHE["nc"]


def _host_prep(x, w_q, w_kv, w_out):
    w_k = np.ascontiguousarray(w_kv[:, :INNER]).astype(BF16)
    w_v = np.ascontiguousarray(w_kv[:, INNER:]).astype(BF16)
    wq16 = np.ascontiguousarray(w_q).astype(BF16)
    wo16 = np.ascontiguousarray(w_out).astype(BF16)
    in_maps = []
    for b in range(B):
        # tokens modality-major: (M, N, D) -> (T, D); transpose to (D, T)
        xb = x[b].transpose(1, 0, 2).reshape(T, D)
        xT = np.ascontiguousarray(xb.T).astype(BF16)
        in_maps.append(
            {"xT": xT, "w_q": wq16, "w_k": w_k, "w_v": w_v, "w_out": wo16}
        )
    return in_maps


def kernel(x, w_q, w_kv, w_out, b_out):
    from concourse.bass_utils import run_bass_kernel_spmd

    nc = _get_nc()
    in_maps = _host_prep(
        np.asarray(x, np.float32),
        np.asarray(w_q, np.float32),
        np.asarray(w_kv, np.float32),
        np.asarray(w_out, np.float32),
    )
    res = run_bass_kernel_spmd(nc, in_maps, core_ids=list(range(NCORES)))
    ys = np.stack([res.results[c]["y"] for c in range(NCORES)], axis=0)
    ys = ys + np.asarray(b_out, np.float32)[None, None, :]
    return ys.reshape(B, N, M, D).astype(np.float32)
